# revision 2
# baseline (speedup 1.0000x reference)
# DCN CrossLayer kernel for Trainium2 (8 NeuronCores, data-parallel over batch).
#
# Reference computation (per example row x of length D, L=3 layers):
#   cross = x
#   for i in range(L):
#       s_i   = <cross, W_i>                  (scalar per example)
#       cross = x * s_i + bias_i + cross
#
# Algebraic collapse: cross_i = a_i * x + B_i with per-example scalar a_i and
# batch-independent vector B_i = sum_{j<i} bias_j.  Then
#   s_i     = a_i * t_i + c_i,   t_i = <x, W_i>,  c_i = <B_i, W_i>
#   a_{i+1} = a_i * (1 + t_i) + c_i
#   out     = a_L * x + B_L
# so the device kernel only needs the three dot products t_i = <x, W_i>,
# a tiny per-row recurrence, and one per-row scale of x.  c_i and B_L are
# computed on the host (they do not depend on the batch).
#
# This version is tuned for the HBM roofline: all device I/O is fp16
# (rel err ~7e-4, well inside the 2e-2 gate), halving HBM traffic vs
# fp32, and x is shipped pre-transposed (d-major) so no on-device
# transpose pass is needed:
#   - host packs x as xt[g, p, k, j] = x[g*G + j, k*128 + p] in fp16
#   - per group g: DMA in xt[g] (contiguous, 4KB per partition line)
#   - PE: t = sum_k wt_k^T @ xs_k accumulated in PSUM.  wt is zero-padded
#     to 65 columns so layer l's dot lands on partition 32*l (compute
#     engines may only address SBUF/PSUM at partition 0/32/64/96).
#   - DVE: u_l = t_l + 1 pulled to partition 0, a3 = ((u0*u1)+c1)*u2+c2
#   - PE: ones-matmul broadcasts a3 to all 128 partitions (PSUM)
#   - DVE: ys = xs * a3_bcast (fp16), DMA out in the same packed layout
#   - host unpacks/upcasts to the (B, D, 1) fp32 result
import os
from contextlib import ExitStack

import numpy as np

import concourse.bacc as bacc
import concourse.bass as bass
import concourse.tile as tile
from concourse import mybir
from concourse.bass_utils import run_bass_kernel_spmd

B, D, L = 16384, 1024, 3
N_CORES = 8
ROWS = B // N_CORES  # rows per core
P = 128
KCH = D // P  # 8 d-chunks of 128
GROUPS = 8
G = ROWS // GROUPS  # 256 rows per group
LPAD = 65  # zero-padded stationary width; layer l at column 32*l

F32 = mybir.dt.float32
F16 = mybir.dt.float16

# test.py can flip these before calling kernel() to get an NTFF profile.
TRACE = False
LAST_RESULT = None


def _build(has_bias: bool, c1: float, c2: float) -> bass.Bass:
    nc = bacc.Bacc("TRN2", target_bir_lowering=False)
    xt = nc.dram_tensor("xt", [GROUPS, P, KCH, G], F16, kind="ExternalInput")
    wt = nc.dram_tensor("wt", [P, KCH, LPAD], F16, kind="ExternalInput")
    if has_bias:
        bt = nc.dram_tensor("bt", [P, KCH], F32, kind="ExternalInput")
    yt = nc.dram_tensor("yt", [GROUPS, P, KCH, G], F16, kind="ExternalOutput")

    with tile.TileContext(nc) as tc, ExitStack() as ctx:
        singles = ctx.enter_context(tc.tile_pool(name="singles", bufs=1))
        xpool = ctx.enter_context(tc.tile_pool(name="xpool", bufs=3))
        ypool = ctx.enter_context(tc.tile_pool(name="ypool", bufs=3))
        small = ctx.enter_context(tc.tile_pool(name="small", bufs=4))
        psT = ctx.enter_context(tc.tile_pool(name="psT", bufs=3, space="PSUM"))
        psB = ctx.enter_context(tc.tile_pool(name="psB", bufs=3, space="PSUM"))

        # tiny constant DMAs go on the SWDGE ring so they cannot delay the
        # first big x in-DMA on the SP HWDGE ring
        wt_sb = singles.tile([P, KCH, LPAD], F16)
        nc.gpsimd.dma_start(out=wt_sb, in_=wt[:])
        ones_sb = singles.tile([1, P], F16)
        nc.vector.memset(ones_sb, 1.0)
        if has_bias:
            bt_sb = singles.tile([P, KCH], F32)
            nc.gpsimd.dma_start(out=bt_sb, in_=bt[:])

        for g in range(GROUPS):
            xs = xpool.tile([P, KCH, G], F16, tag="xs")
            nc.sync.dma_start(out=xs, in_=xt[g])
            # t[32*l, j] = sum_d x[j, d] * W[l, d], accumulated over d-chunks
            pt = psT.tile([LPAD, G], F32)
            for k in range(KCH):
                nc.tensor.matmul(
                    pt,
                    wt_sb[:, k, :],
                    xs[:, k, :],
                    start=(k == 0),
                    stop=(k == KCH - 1),
                )
            # u_l = 1 + t_l, pulled down to partition 0
            ua = small.tile([1, G], F32, tag="ua")
            ub = small.tile([1, G], F32, tag="ub")
            uc = small.tile([1, G], F32, tag="uc")
            nc.vector.tensor_scalar_add(ua, pt[0:1, :], 1.0)
            nc.vector.tensor_scalar_add(ub, pt[32:33, :], 1.0)
            nc.vector.tensor_scalar_add(uc, pt[64:65, :], 1.0)
            # a3 = ((u0*u1)+c1)*u2 + c2
            a = small.tile([1, G], F32, tag="a")
            nc.vector.tensor_mul(a, ua, ub)
            if c1 != 0.0:
                nc.vector.tensor_scalar_add(a, a, c1)
            nc.vector.tensor_mul(a, a, uc)
            if c2 != 0.0:
                nc.vector.tensor_scalar_add(a, a, c2)
            ah = small.tile([1, G], F16, tag="ah")
            nc.scalar.copy(out=ah, in_=a)
            # broadcast a3 across partitions: ones[1,P].T @ ah[1,G] -> [P,G]
            pb = psB.tile([P, G], F32)
            nc.tensor.matmul(pb, ones_sb, ah)
            pbh = small.tile([P, G], F16, tag="pbh")
            nc.scalar.copy(out=pbh, in_=pb)
            # ys = xs * a3 (+ B_L), a3 broadcast over the chunk dim (stride 0)
            ys = ypool.tile([P, KCH, G], F16, tag="ys")
            pb_b = bass.AP(
                tensor=pbh.tensor,
                offset=pbh.offset,
                ap=[pbh.ap[0], [0, KCH], pbh.ap[1]],
            )
            nc.vector.tensor_mul(ys, xs, pb_b)
            if has_bias:
                for k in range(KCH):
                    nc.vector.tensor_scalar_add(
                        ys[:, k, :], ys[:, k, :], bt_sb[:, k : k + 1]
                    )
            # out-DMA on the ACT HWDGE ring so it can't FIFO-block in-DMAs
            nc.scalar.dma_start(out=yt[g], in_=ys)
    nc.finalize()
    return nc


def kernel(x, W, bias):
    global LAST_RESULT
    x2 = np.asarray(x, dtype=np.float32).reshape(B, D)
    W2 = np.asarray(W, dtype=np.float32).reshape(L, D)
    B2 = np.asarray(bias, dtype=np.float32).reshape(L, D)

    # host-side constants
    has_bias = bool(np.any(B2 != 0.0))
    c1 = float(B2[0] @ W2[1])
    c2 = float((B2[0] + B2[1]) @ W2[2])
    b3 = B2.sum(axis=0)
    # wt[p, k, 32*l] = W[l, k*128 + p], zero elsewhere
    wt_host = np.zeros((P, KCH, LPAD), dtype=np.float16)
    wt_host[:, :, ::32] = W2.T.reshape(KCH, P, L).transpose(1, 0, 2)
    # bt[p, k] = B_L[k*128 + p]
    bt_host = np.ascontiguousarray(b3.reshape(KCH, P).T)

    nc = _build(has_bias, c1 if has_bias else 0.0, c2 if has_bias else 0.0)

    in_maps = []
    for c in range(N_CORES):
        xc = x2[c * ROWS : (c + 1) * ROWS]
        # xt[g, p, k, j] = xc[g*G + j, k*128 + p]
        xt_host = np.ascontiguousarray(
            xc.reshape(GROUPS, G, KCH, P).transpose(0, 3, 2, 1).astype(np.float16)
        )
        m = {"xt": xt_host, "wt": wt_host}
        if has_bias:
            m["bt"] = bt_host
        in_maps.append(m)

    kwargs = {}
    if TRACE:
        kwargs = dict(trace=True, trace_cores=[0])
    res = run_bass_kernel_spmd(nc, in_maps, core_ids=list(range(N_CORES)), **kwargs)
    LAST_RESULT = res
    out = np.empty((B, D), dtype=np.float32)
    for c in range(N_CORES):
        yt = res.results[c]["yt"]  # [GROUPS, P, KCH, G] fp16
        out[c * ROWS : (c + 1) * ROWS] = (
            yt.transpose(0, 3, 2, 1).reshape(ROWS, D).astype(np.float32)
        )
    return np.ascontiguousarray(out.reshape(B, D, 1))


# revision 3
# speedup vs baseline: 1.1456x; 1.1456x over previous
# DCN CrossLayer kernel for Trainium2 (8 NeuronCores, data-parallel over batch).
#
# Reference computation (per example row x of length D, L=3 layers):
#   cross = x
#   for i in range(L):
#       s_i   = <cross, W_i>                  (scalar per example)
#       cross = x * s_i + bias_i + cross
#
# Algebraic collapse: cross_i = a_i * x + B_i with per-example scalar a_i and
# batch-independent vector B_i = sum_{j<i} bias_j.  Then
#   s_i     = a_i * t_i + c_i,   t_i = <x, W_i>,  c_i = <B_i, W_i>
#   a_{i+1} = a_i * (1 + t_i) + c_i
#   out     = a_L * x + B_L
# so the device kernel only needs the three dot products t_i = <x, W_i>,
# a tiny per-row recurrence, and one per-row scale of x.  c_i and B_L are
# computed on the host (they do not depend on the batch).
#
# Tuned for the HBM roofline AND pipeline latency:
#   - all device I/O is fp16 (rel err ~8e-4, inside the 2e-2 gate): 4 MiB
#     in + 4 MiB out per core
#   - x ships pre-transposed (d on partitions) so no on-device transpose:
#     host packs xt[g, p, k, j] = x[g*G + j, k*128 + p]
#   - PE: U_l = 1 + <x, W_l> accumulated in PSUM; wt is zero-padded to 65
#     columns so layer l lands on partition 32*l (compute engines may only
#     address partitions 0/32/64/96), and a final ones-column matmul adds
#     the +1 so no DVE adds are needed
#   - recurrence: one ACT pull of U_0 plus two DVE muls reading U_1/U_2
#     straight out of PSUM at bases 32/64 (one PSUM operand per op is legal)
#   - a3 broadcast to 128 partitions via gpsimd.partition_broadcast --
#     keeps the PE stream free of backward (PE<-DVE) dependencies
#   - ys = xs * a3 on DVE (fp16), emitted with a one-group skew so no
#     engine stream ever waits on same-group cross-engine results
import os
from contextlib import ExitStack

import numpy as np

import concourse.bacc as bacc
import concourse.bass as bass
import concourse.tile as tile
from concourse import mybir
from concourse.bass_utils import run_bass_kernel_spmd

B, D, L = 16384, 1024, 3
N_CORES = 8
ROWS = B // N_CORES  # rows per core
P = 128
KCH = D // P  # 8 d-chunks of 128
GROUPS = 8
G = ROWS // GROUPS  # 256 rows per group
LPAD = 65  # zero-padded stationary width; layer l at column 32*l

F32 = mybir.dt.float32
F16 = mybir.dt.float16

# test.py can flip these before calling kernel() to get an NTFF profile.
TRACE = False
LAST_RESULT = None


def _build(has_bias: bool, c1: float, c2: float) -> bass.Bass:
    nc = bacc.Bacc("TRN2", target_bir_lowering=False)
    xt = nc.dram_tensor("xt", [GROUPS, P, KCH, G], F16, kind="ExternalInput")
    wt = nc.dram_tensor("wt", [P, KCH, LPAD], F16, kind="ExternalInput")
    w1 = nc.dram_tensor("w1", [1, LPAD], F16, kind="ExternalInput")
    if has_bias:
        bt = nc.dram_tensor("bt", [P, KCH], F32, kind="ExternalInput")
    yt = nc.dram_tensor("yt", [GROUPS, P, KCH, G], F16, kind="ExternalOutput")

    with tile.TileContext(nc) as tc, ExitStack() as ctx:
        singles = ctx.enter_context(tc.tile_pool(name="singles", bufs=1))
        xpool = ctx.enter_context(tc.tile_pool(name="xpool", bufs=4))
        ypool = ctx.enter_context(tc.tile_pool(name="ypool", bufs=3))
        small = ctx.enter_context(tc.tile_pool(name="small", bufs=3))
        psT = ctx.enter_context(tc.tile_pool(name="psT", bufs=3, space="PSUM"))

        # tiny constant DMAs go on the SWDGE ring so they cannot delay the
        # first big x in-DMA on the SP HWDGE ring
        wt_sb = singles.tile([P, KCH, LPAD], F16)
        nc.gpsimd.dma_start(out=wt_sb, in_=wt[:])
        w1_sb = singles.tile([1, LPAD], F16)
        nc.gpsimd.dma_start(out=w1_sb, in_=w1[:])
        one_row = singles.tile([1, G], F16)
        nc.vector.memset(one_row, 1.0)
        if has_bias:
            bt_sb = singles.tile([P, KCH], F32)
            nc.gpsimd.dma_start(out=bt_sb, in_=bt[:])

        xs_t = [None] * GROUPS
        pbh_t = [None] * GROUPS
        for g in range(GROUPS + 1):
            if g >= 1:
                # skewed tail of group g-1: scale + store
                h = g - 1
                ys = ypool.tile([P, KCH, G], F16, tag="ys")
                pbh = pbh_t[h]
                pb_b = bass.AP(
                    tensor=pbh.tensor,
                    offset=pbh.offset,
                    ap=[pbh.ap[0], [0, KCH], pbh.ap[1]],
                )
                nc.vector.tensor_mul(ys, xs_t[h], pb_b)
                if has_bias:
                    for k in range(KCH):
                        nc.vector.tensor_scalar_add(
                            ys[:, k, :], ys[:, k, :], bt_sb[:, k : k + 1]
                        )
                # out-DMA on the ACT HWDGE ring so it can't FIFO-block in-DMAs
                nc.scalar.dma_start(out=yt[h], in_=ys)
            if g >= GROUPS:
                break
            xs = xpool.tile([P, KCH, G], F16, tag="xs")
            xs_t[g] = xs
            nc.sync.dma_start(out=xs, in_=xt[g])
            # U[32*l, j] = 1 + sum_d x[j, d] * W[l, d]
            pt = psT.tile([LPAD, G], F32)
            for k in range(KCH):
                nc.tensor.matmul(
                    pt, wt_sb[:, k, :], xs[:, k, :], start=(k == 0), stop=False
                )
            nc.tensor.matmul(pt, w1_sb, one_row, start=False, stop=True)
            # a3 = ((U0*U1)+c1)*U2 + c2  (c1 = c2 = 0 when bias is zero)
            ua = small.tile([1, G], F32, tag="ua")
            nc.scalar.copy(out=ua, in_=pt[0:1, :])
            a = small.tile([1, G], F32, tag="a")
            nc.vector.tensor_mul(a, ua, pt[32:33, :])
            if c1 != 0.0:
                nc.vector.tensor_scalar_add(a, a, c1)
            ah = small.tile([1, G], F16, tag="ah")
            if c2 != 0.0:
                a2 = small.tile([1, G], F32, tag="a2")
                nc.vector.tensor_mul(a2, a, pt[64:65, :])
                nc.vector.tensor_scalar_add(a2, a2, c2)
                nc.scalar.copy(out=ah, in_=a2)
            else:
                nc.vector.tensor_mul(ah, a, pt[64:65, :])
            pbh = small.tile([P, G], F16, tag="pbh")
            pbh_t[g] = pbh
            nc.gpsimd.partition_broadcast(pbh, ah)
    nc.finalize()
    return nc


def kernel(x, W, bias):
    global LAST_RESULT
    x2 = np.asarray(x, dtype=np.float32).reshape(B, D)
    W2 = np.asarray(W, dtype=np.float32).reshape(L, D)
    B2 = np.asarray(bias, dtype=np.float32).reshape(L, D)

    # host-side constants
    has_bias = bool(np.any(B2 != 0.0))
    c1 = float(B2[0] @ W2[1])
    c2 = float((B2[0] + B2[1]) @ W2[2])
    b3 = B2.sum(axis=0)
    # wt[p, k, 32*l] = W[l, k*128 + p], zero elsewhere
    wt_host = np.zeros((P, KCH, LPAD), dtype=np.float16)
    wt_host[:, :, ::32] = W2.T.reshape(KCH, P, L).transpose(1, 0, 2)
    w1_host = np.zeros((1, LPAD), dtype=np.float16)
    w1_host[0, ::32] = 1.0
    # bt[p, k] = B_L[k*128 + p]
    bt_host = np.ascontiguousarray(b3.reshape(KCH, P).T)

    nc = _build(has_bias, c1 if has_bias else 0.0, c2 if has_bias else 0.0)

    in_maps = []
    for c in range(N_CORES):
        xc = x2[c * ROWS : (c + 1) * ROWS]
        # xt[g, p, k, j] = xc[g*G + j, k*128 + p]
        xt_host = np.ascontiguousarray(
            xc.reshape(GROUPS, G, KCH, P).transpose(0, 3, 2, 1).astype(np.float16)
        )
        m = {"xt": xt_host, "wt": wt_host, "w1": w1_host}
        if has_bias:
            m["bt"] = bt_host
        in_maps.append(m)

    kwargs = {}
    if TRACE:
        kwargs = dict(trace=True, trace_cores=[0])
    res = run_bass_kernel_spmd(nc, in_maps, core_ids=list(range(N_CORES)), **kwargs)
    LAST_RESULT = res
    out = np.empty((B, D), dtype=np.float32)
    for c in range(N_CORES):
        yt = res.results[c]["yt"]  # [GROUPS, P, KCH, G] fp16
        out[c * ROWS : (c + 1) * ROWS] = (
            yt.transpose(0, 3, 2, 1).reshape(ROWS, D).astype(np.float32)
        )
    return np.ascontiguousarray(out.reshape(B, D, 1))


# revision 4
# speedup vs baseline: 1.1911x; 1.0398x over previous
# DCN CrossLayer kernel for Trainium2 (8 NeuronCores, data-parallel over batch).
#
# Reference computation (per example row x of length D, L=3 layers):
#   cross = x
#   for i in range(L):
#       s_i   = <cross, W_i>                  (scalar per example)
#       cross = x * s_i + bias_i + cross
#
# Algebraic collapse: cross_i = a_i * x + B_i with per-example scalar a_i and
# batch-independent vector B_i = sum_{j<i} bias_j.  Then
#   s_i     = a_i * t_i + c_i,   t_i = <x, W_i>,  c_i = <B_i, W_i>
#   a_{i+1} = a_i * (1 + t_i) + c_i
#   out     = a_L * x + B_L
# so the device kernel only needs the three dot products t_i = <x, W_i>,
# a tiny per-row recurrence, and one per-row scale of x.  c_i and B_L are
# computed on the host (they do not depend on the batch).
#
# Tuned for the HBM roofline AND pipeline latency:
#   - all device I/O is fp16 (rel err ~8e-4, inside the 2e-2 gate): 4 MiB
#     in + 4 MiB out per core
#   - x ships pre-transposed (d on partitions) so no on-device transpose:
#     host packs xt[g, p, k, j] = x[g*G + j, k*128 + p]
#   - PE: U_l = 1 + <x, W_l> accumulated in PSUM; wt is zero-padded to 65
#     columns so layer l lands on partition 32*l (compute engines may only
#     address partitions 0/32/64/96), and a final ones-column matmul adds
#     the +1 so no DVE adds are needed
#   - recurrence: one ACT pull of U_0 plus two DVE muls reading U_1/U_2
#     straight out of PSUM at bases 32/64 (one PSUM operand per op is legal)
#   - a3 broadcast to 128 partitions via gpsimd.partition_broadcast --
#     keeps the PE stream free of backward (PE<-DVE) dependencies
#   - ys = xs * a3 on DVE (fp16), emitted with a one-group skew so no
#     engine stream ever waits on same-group cross-engine results
import os
from contextlib import ExitStack

import numpy as np

import concourse.bacc as bacc
import concourse.bass as bass
import concourse.tile as tile
from concourse import mybir
from concourse.bass_utils import run_bass_kernel_spmd

B, D, L = 16384, 1024, 3
N_CORES = 8
ROWS = B // N_CORES  # rows per core
P = 128
KCH = D // P  # 8 d-chunks of 128
GROUPS = 4
G = ROWS // GROUPS  # 256 rows per group
LPAD = 65  # zero-padded stationary width; layer l at column 32*l

F32 = mybir.dt.float32
F16 = mybir.dt.float16

# test.py can flip these before calling kernel() to get an NTFF profile.
TRACE = False
LAST_RESULT = None


def _build(has_bias: bool, c1: float, c2: float) -> bass.Bass:
    nc = bacc.Bacc("TRN2", target_bir_lowering=False)
    xt = nc.dram_tensor("xt", [GROUPS, P, KCH, G], F16, kind="ExternalInput")
    wt = nc.dram_tensor("wt", [P, KCH, LPAD], F16, kind="ExternalInput")
    if has_bias:
        bt = nc.dram_tensor("bt", [P, KCH], F32, kind="ExternalInput")
    yt = nc.dram_tensor("yt", [GROUPS, P, KCH, G], F16, kind="ExternalOutput")

    with tile.TileContext(nc) as tc, ExitStack() as ctx:
        singles = ctx.enter_context(tc.tile_pool(name="singles", bufs=1))
        xpool = ctx.enter_context(tc.tile_pool(name="xpool", bufs=4))
        ypool = ctx.enter_context(tc.tile_pool(name="ypool", bufs=3))
        small = ctx.enter_context(tc.tile_pool(name="small", bufs=3))
        psT = ctx.enter_context(tc.tile_pool(name="psT", bufs=3, space="PSUM"))

        # tiny constant DMAs go on the SWDGE ring so they cannot delay the
        # first big x in-DMA on the SP HWDGE ring
        wt_sb = singles.tile([P, KCH, LPAD], F16)
        nc.gpsimd.dma_start(out=wt_sb, in_=wt[:])
        if has_bias:
            bt_sb = singles.tile([P, KCH], F32)
            nc.gpsimd.dma_start(out=bt_sb, in_=bt[:])

        xs_t = [None] * GROUPS
        pbh_t = [None] * GROUPS
        for g in range(GROUPS + 1):
            if g >= 1:
                # skewed tail of group g-1: scale + store
                h = g - 1
                ys = ypool.tile([P, KCH, G], F16, tag="ys")
                pbh = pbh_t[h]
                pb_b = bass.AP(
                    tensor=pbh.tensor,
                    offset=pbh.offset,
                    ap=[pbh.ap[0], [0, KCH], pbh.ap[1]],
                )
                nc.vector.tensor_mul(ys, xs_t[h], pb_b)
                if has_bias:
                    for k in range(KCH):
                        nc.vector.tensor_scalar_add(
                            ys[:, k, :], ys[:, k, :], bt_sb[:, k : k + 1]
                        )
                # out-DMA on the ACT HWDGE ring so it can't FIFO-block in-DMAs
                nc.scalar.dma_start(out=yt[h], in_=ys)
            if g >= GROUPS:
                break
            xs = xpool.tile([P, KCH, G], F16, tag="xs")
            xs_t[g] = xs
            nc.sync.dma_start(out=xs, in_=xt[g])
            # t[32*l, j] = sum_d x[j, d] * W[l, d]
            pt = psT.tile([LPAD, G], F32)
            for k in range(KCH):
                nc.tensor.matmul(
                    pt, wt_sb[:, k, :], xs[:, k, :], start=(k == 0), stop=(k == KCH - 1)
                )
            # U_l = 1 + t_l pulled to partition 0 on ACT (Copy applies
            # scale*in + bias before the identity function)
            CP = mybir.ActivationFunctionType.Copy
            ua = small.tile([1, G], F32, tag="ua")
            ub = small.tile([1, G], F32, tag="ub")
            uc = small.tile([1, G], F32, tag="uc")
            nc.scalar.activation(ua, pt[0:1, :], CP, bias=1.0)
            nc.scalar.activation(ub, pt[32:33, :], CP, bias=1.0)
            nc.scalar.activation(uc, pt[64:65, :], CP, bias=1.0)
            # a3 = ((U0*U1)+c1)*U2 + c2  (c1 = c2 = 0 when bias is zero)
            a = small.tile([1, G], F32, tag="a")
            nc.vector.tensor_mul(a, ua, ub)
            if c1 != 0.0:
                nc.vector.tensor_scalar_add(a, a, c1)
            ah = small.tile([1, G], F16, tag="ah")
            if c2 != 0.0:
                a2 = small.tile([1, G], F32, tag="a2")
                nc.vector.tensor_mul(a2, a, uc)
                nc.vector.tensor_scalar_add(a2, a2, c2)
                nc.scalar.copy(out=ah, in_=a2)
            else:
                nc.vector.tensor_mul(ah, a, uc)
            pbh = small.tile([P, G], F16, tag="pbh")
            pbh_t[g] = pbh
            nc.gpsimd.partition_broadcast(pbh, ah)
    nc.finalize()
    return nc


def kernel(x, W, bias):
    global LAST_RESULT
    x2 = np.asarray(x, dtype=np.float32).reshape(B, D)
    W2 = np.asarray(W, dtype=np.float32).reshape(L, D)
    B2 = np.asarray(bias, dtype=np.float32).reshape(L, D)

    # host-side constants
    has_bias = bool(np.any(B2 != 0.0))
    c1 = float(B2[0] @ W2[1])
    c2 = float((B2[0] + B2[1]) @ W2[2])
    b3 = B2.sum(axis=0)
    # wt[p, k, 32*l] = W[l, k*128 + p], zero elsewhere
    wt_host = np.zeros((P, KCH, LPAD), dtype=np.float16)
    wt_host[:, :, ::32] = W2.T.reshape(KCH, P, L).transpose(1, 0, 2)
    # bt[p, k] = B_L[k*128 + p]
    bt_host = np.ascontiguousarray(b3.reshape(KCH, P).T)

    nc = _build(has_bias, c1 if has_bias else 0.0, c2 if has_bias else 0.0)

    in_maps = []
    for c in range(N_CORES):
        xc = x2[c * ROWS : (c + 1) * ROWS]
        # xt[g, p, k, j] = xc[g*G + j, k*128 + p]
        xt_host = np.ascontiguousarray(
            xc.reshape(GROUPS, G, KCH, P).transpose(0, 3, 2, 1).astype(np.float16)
        )
        m = {"xt": xt_host, "wt": wt_host}
        if has_bias:
            m["bt"] = bt_host
        in_maps.append(m)

    kwargs = {}
    if TRACE:
        kwargs = dict(trace=True, trace_cores=[0])
    res = run_bass_kernel_spmd(nc, in_maps, core_ids=list(range(N_CORES)), **kwargs)
    LAST_RESULT = res
    out = np.empty((B, D), dtype=np.float32)
    for c in range(N_CORES):
        yt = res.results[c]["yt"]  # [GROUPS, P, KCH, G] fp16
        out[c * ROWS : (c + 1) * ROWS] = (
            yt.transpose(0, 3, 2, 1).reshape(ROWS, D).astype(np.float32)
        )
    return np.ascontiguousarray(out.reshape(B, D, 1))


# revision 5
# speedup vs baseline: 1.2571x; 1.0554x over previous
# DCN CrossLayer kernel for Trainium2 (8 NeuronCores, data-parallel over batch).
#
# Reference computation (per example row x of length D, L=3 layers):
#   cross = x
#   for i in range(L):
#       s_i   = <cross, W_i>                  (scalar per example)
#       cross = x * s_i + bias_i + cross
#
# Algebraic collapse: cross_i = a_i * x + B_i with per-example scalar a_i and
# batch-independent vector B_i = sum_{j<i} bias_j.  Then
#   s_i     = a_i * t_i + c_i,   t_i = <x, W_i>,  c_i = <B_i, W_i>
#   a_{i+1} = a_i * (1 + t_i) + c_i
#   out     = a_L * x + B_L
# so the device kernel only needs the three dot products t_i = <x, W_i>,
# a tiny per-row recurrence, and one per-row scale of x.  c_i and B_L are
# computed on the host (they do not depend on the batch).
#
# Measured HW facts this version is tuned around (perfetto):
#   - 16 DMA queues x ~25 GB/s each => ~400 GB/s/core aggregate; descriptor
#     cost scales with bytes, so fp16 I/O (4+4 MiB/core) floors DMA at ~21us
#   - PE matmul cost is ~flat (~215ns + 152ns ldweights) for N <= 512, so
#     dot-groups are G=512 rows (4 groups)
#   - concurrently-enqueued DMAs round-robin-share the queues, so in-DMAs
#     are split per 4-chunk half to let the first matmuls start early
#   - small DVE/ACT ops cost 0.3-0.7us each regardless of size: the +1s are
#     folded into ACT pulls (Copy applies scale*in+bias), recurrence is 2
#     DVE muls reading PSUM at partition 32/64 (quadrant rule), and a3 is
#     partition-broadcast on gpsimd -- PE stream has no backward deps
#   - ys/out are emitted per 256-row half with a one-group skew so output
#     DMAs interleave with remaining input DMAs instead of trailing them
import os
from contextlib import ExitStack

import numpy as np

import concourse.bacc as bacc
import concourse.bass as bass
import concourse.tile as tile
from concourse import mybir
from concourse.bass_utils import run_bass_kernel_spmd

B, D, L = 16384, 1024, 3
N_CORES = 8
ROWS = B // N_CORES  # rows per core
P = 128
KCH = D // P  # 8 d-chunks of 128
GROUPS = 4
G = ROWS // GROUPS  # 512 rows per dot-group
H = G // 2  # 256-row halves for the scale/store stage
LPAD = 65  # zero-padded stationary width; layer l at column 32*l

F32 = mybir.dt.float32
F16 = mybir.dt.float16

# test.py can flip these before calling kernel() to get an NTFF profile.
TRACE = False
LAST_RESULT = None


def _build(has_bias: bool, c1: float, c2: float) -> bass.Bass:
    nc = bacc.Bacc("TRN2", target_bir_lowering=False)
    xt = nc.dram_tensor("xt", [GROUPS, P, KCH, G], F16, kind="ExternalInput")
    wt = nc.dram_tensor("wt", [P, KCH, LPAD], F16, kind="ExternalInput")
    if has_bias:
        bt = nc.dram_tensor("bt", [P, KCH], F32, kind="ExternalInput")
    yt = nc.dram_tensor("yt", [GROUPS, 2, P, KCH, H], F16, kind="ExternalOutput")

    with tile.TileContext(nc) as tc, ExitStack() as ctx:
        singles = ctx.enter_context(tc.tile_pool(name="singles", bufs=1))
        xpool = ctx.enter_context(tc.tile_pool(name="xpool", bufs=3))
        ypool = ctx.enter_context(tc.tile_pool(name="ypool", bufs=4))
        small = ctx.enter_context(tc.tile_pool(name="small", bufs=3))
        bpool = ctx.enter_context(tc.tile_pool(name="bpool", bufs=4))
        psT = ctx.enter_context(tc.tile_pool(name="psT", bufs=2, space="PSUM"))

        # tiny constant DMA goes on the SWDGE ring so it cannot delay the
        # first big x in-DMA on the SP HWDGE ring
        wt_sb = singles.tile([P, KCH, LPAD], F16)
        nc.gpsimd.dma_start(out=wt_sb, in_=wt[:])
        if has_bias:
            bt_sb = singles.tile([P, KCH], F32)
            nc.gpsimd.dma_start(out=bt_sb, in_=bt[:])

        CP = mybir.ActivationFunctionType.Copy
        KH = KCH // 2
        xs_t = [None] * GROUPS
        pbh_t = [None] * GROUPS
        for g in range(GROUPS + 1):
            if g >= 1:
                # skewed tail of group g-1: scale + store per 256-row half
                hg = g - 1
                for h in range(2):
                    ys = ypool.tile([P, KCH, H], F16, tag="ys")
                    pbh = pbh_t[hg][h]
                    pb_b = bass.AP(
                        tensor=pbh.tensor,
                        offset=pbh.offset,
                        ap=[pbh.ap[0], [0, KCH], pbh.ap[1]],
                    )
                    nc.vector.tensor_mul(
                        ys, xs_t[hg][:, :, h * H : (h + 1) * H], pb_b
                    )
                    if has_bias:
                        for k in range(KCH):
                            nc.vector.tensor_scalar_add(
                                ys[:, k, :], ys[:, k, :], bt_sb[:, k : k + 1]
                            )
                    # out-DMA on the ACT HWDGE ring
                    nc.scalar.dma_start(out=yt[hg, h], in_=ys)
            if g >= GROUPS:
                break
            xs = xpool.tile([P, KCH, G], F16, tag="xs")
            xs_t[g] = xs
            # split per 4-chunk half so the first matmuls can start after
            # only half the group has landed
            nc.sync.dma_start(out=xs[:, 0:KH, :], in_=xt[g, :, 0:KH, :])
            nc.sync.dma_start(out=xs[:, KH:KCH, :], in_=xt[g, :, KH:KCH, :])
            # t[32*l, j] = sum_d x[j, d] * W[l, d]
            pt = psT.tile([LPAD, G], F32)
            for k in range(KCH):
                nc.tensor.matmul(
                    pt, wt_sb[:, k, :], xs[:, k, :], start=(k == 0), stop=(k == KCH - 1)
                )
            # U_l = 1 + t_l pulled to partition 0 on ACT (Copy applies
            # scale*in + bias before the identity function)
            ua = small.tile([1, G], F32, tag="ua")
            ub = small.tile([1, G], F32, tag="ub")
            uc = small.tile([1, G], F32, tag="uc")
            nc.scalar.activation(ua, pt[0:1, :], CP, bias=1.0)
            nc.scalar.activation(ub, pt[32:33, :], CP, bias=1.0)
            nc.scalar.activation(uc, pt[64:65, :], CP, bias=1.0)
            # a3 = ((U0*U1)+c1)*U2 + c2  (c1 = c2 = 0 when bias is zero)
            a = small.tile([1, G], F32, tag="a")
            nc.vector.tensor_mul(a, ua, ub)
            if c1 != 0.0:
                nc.vector.tensor_scalar_add(a, a, c1)
            ah = small.tile([1, G], F16, tag="ah")
            if c2 != 0.0:
                a2 = small.tile([1, G], F32, tag="a2")
                nc.vector.tensor_mul(a2, a, uc)
                nc.vector.tensor_scalar_add(a2, a2, c2)
                nc.scalar.copy(out=ah, in_=a2)
            else:
                nc.vector.tensor_mul(ah, a, uc)
            # broadcast a3 to all partitions, per half (gpsimd, off the PE)
            pbh_t[g] = []
            for h in range(2):
                pbh = bpool.tile([P, H], F16, tag="pbh")
                pbh_t[g].append(pbh)
                nc.gpsimd.partition_broadcast(pbh, ah[0:1, h * H : (h + 1) * H])
    nc.finalize()
    return nc


def kernel(x, W, bias):
    global LAST_RESULT
    x2 = np.asarray(x, dtype=np.float32).reshape(B, D)
    W2 = np.asarray(W, dtype=np.float32).reshape(L, D)
    B2 = np.asarray(bias, dtype=np.float32).reshape(L, D)

    # host-side constants
    has_bias = bool(np.any(B2 != 0.0))
    c1 = float(B2[0] @ W2[1])
    c2 = float((B2[0] + B2[1]) @ W2[2])
    b3 = B2.sum(axis=0)
    # wt[p, k, 32*l] = W[l, k*128 + p], zero elsewhere
    wt_host = np.zeros((P, KCH, LPAD), dtype=np.float16)
    wt_host[:, :, ::32] = W2.T.reshape(KCH, P, L).transpose(1, 0, 2)
    # bt[p, k] = B_L[k*128 + p]
    bt_host = np.ascontiguousarray(b3.reshape(KCH, P).T)

    nc = _build(has_bias, c1 if has_bias else 0.0, c2 if has_bias else 0.0)

    in_maps = []
    for c in range(N_CORES):
        xc = x2[c * ROWS : (c + 1) * ROWS]
        # xt[g, p, k, j] = xc[g*G + j, k*128 + p]
        xt_host = np.ascontiguousarray(
            xc.reshape(GROUPS, G, KCH, P).transpose(0, 3, 2, 1).astype(np.float16)
        )
        m = {"xt": xt_host, "wt": wt_host}
        if has_bias:
            m["bt"] = bt_host
        in_maps.append(m)

    kwargs = {}
    if TRACE:
        kwargs = dict(trace=True, trace_cores=[0])
    res = run_bass_kernel_spmd(nc, in_maps, core_ids=list(range(N_CORES)), **kwargs)
    LAST_RESULT = res
    out = np.empty((B, D), dtype=np.float32)
    for c in range(N_CORES):
        yt = res.results[c]["yt"]  # [GROUPS, 2, P, KCH, H] fp16
        # y[g*G + h*H + j, k*128 + p] = yt[g, h, p, k, j]
        out[c * ROWS : (c + 1) * ROWS] = (
            yt.transpose(0, 1, 4, 3, 2).reshape(ROWS, D).astype(np.float32)
        )
    return np.ascontiguousarray(out.reshape(B, D, 1))
